# revision 14
# baseline (speedup 1.0000x reference)
"""KMeans inference (argmin over squared distances) on 8 Trainium2 cores.

Problem: features [262144, 768] fp32, cluster_centers [1024, 768] fp32.
Output: argmin_k ||x_i - c_k||^2 as int32 [262144].

Strategy (data-parallel over rows, fp8 scoring + exact host refinement):
  - argmin_k ||x-c_k||^2 == argmax_k (x.c_k - 0.5*||c_k||^2); the ||x||^2
    term is constant per row and drops out of the argmin.
  - Shard rows across 8 cores (32768 rows/core). Host pre-transposes each
    shard to xT [768, 32768] and quantizes to fp8 e4m3 (TRN FP8_EXP4 ==
    ml_dtypes.float8_e4m3; |x| ~ N(0,1) is far inside the +-240 range).
  - Per core: scores[m, k] = sum_d xT[d, m] * cT[d, k] via PE matmuls in
    fp8 with perf_mode=DoubleRow (2 fp8 weights per PE cell, contraction
    256 per matmul => 2x FLOP rate vs bf16/fp32r). d=768 = 3 chunks of
    256; both packed operands use the same d -> (chunk, pair, partition)
    layout so the contraction is consistent.
  - Clusters are pre-sorted by ||c||^2 on the host, so each of 32 segments
    of 32 clusters has a tight bias range [bmin_s, bmax_s]. The device
    exports only 32 raw (bias-free) per-segment score maxes per row: one
    DVE segmented reduce straight out of each PSUM bank, fp32 -> fp16.
    No scalar-engine copies, no bias add, no argmax index work on-device.
  - Host: segment s can contain the winner iff segraw_s + bmax_s >=
    max_s'(segraw_s' + bmin_s') - T. Rescore all such segments exactly
    (fp32 BLAS, grouped per segment) and take the exact argmax. fp8 score
    noise has sigma ~1.0, so T=5 makes a missed true-argmax essentially
    impossible (calibrated: 0 mismatches of 262144 at T>=4; ~2.3 segments
    rescored per row).
"""

import sys

sys.path.insert(0, "/opt/trn_rl_repo")

import numpy as np

N_CORES = 8
N, K, D = 262144, 1024, 768
ROWS_PER_CORE = N // N_CORES           # 32768
SLAB_ROWS = 1024                       # rows fetched per DMA slab
N_SLABS = ROWS_PER_CORE // SLAB_ROWS   # 32
SUBTILES = SLAB_ROWS // 128            # 8 row-tiles of 128 per slab
N_ROWTILES = ROWS_PER_CORE // 128      # 256
DCH = 3                                # d-chunks of 256 (DoubleRow pairs)
SEG = 32                               # exported segment maxes per row
SEGW = K // SEG                        # 32 clusters per segment
SEG_BANK = SEG // 2                    # segments per PSUM bank
OUT_CHUNK_SLABS = 1                    # stage flushed every slab

# fp8 e4m3 quantization of x and c gives score noise sigma ~1.0; the fp16
# segmax rounding adds <~0.07.  A missed true-argmax needs noise-diff > T
# across a segment boundary.  T=5 calibrates to 0/262144 mismatches while
# rescoring ~2.3 segments/row on the host.
GAP_THRESHOLD = 5.0

_PROGRAM = None


def _build_program():
    import concourse.mybir as mybir
    from concourse import bacc
    from concourse.tile import TileContext

    F32 = mybir.dt.float32
    F16 = mybir.dt.float16
    F8 = mybir.dt.float8e4

    nc = bacc.Bacc()
    # Inputs (per core): fp8-packed transposed feature shard + centroids
    # (row d ordering: d = t*256 + i*128 + p for chunk t, pair i, part p;
    # clusters pre-permuted by ||c||^2 on the host).
    xt = nc.declare_dram_parameter("xt", [D, ROWS_PER_CORE], F8, isOutput=False)
    cbt = nc.declare_dram_parameter("cbt", [D, K], F8, isOutput=False)
    # Output: seg[p, mc*SEG + s] = max_{k in seg s} x_row . c_k  (raw, no
    # bias) for row mc*128 + p.
    out_seg = nc.declare_dram_parameter(
        "seg", [128, SEG * N_ROWTILES], F16, isOutput=True
    )

    with TileContext(nc) as tc:
        with (
            tc.tile_pool(name="sbuf", bufs=1) as sbuf_pool,
            tc.tile_pool(name="psum", bufs=4, space="PSUM") as psum_pool,
        ):
            consts = xslab_pool = stage_pool = sbuf_pool
            cbt_r = cbt.rearrange("(t i p) k -> p t i k", p=128, i=2)
            xt_r = xt.rearrange("(t i p) r -> p t i r", p=128, i=2)

            # Centroids resident in SBUF, DoubleRow-packed: [128, 3, 2, 1024].
            # Split across both HWDGE queues (sync + scalar) by k-half so the
            # startup-critical load finishes in half the time.
            cb = consts.tile([128, DCH, 2, K], F8, tag="cb")
            nc.sync.dma_start(out=cb[:, :, :, 0:512], in_=cbt_r[:, :, :, 0:512])
            nc.scalar.dma_start(out=cb[:, :, :, 512:1024], in_=cbt_r[:, :, :, 512:1024])

            chunk_rt = OUT_CHUNK_SLABS * SUBTILES  # 16 row-tiles per chunk
            staging = None

            for slab in range(N_SLABS):
                r0 = slab * SLAB_ROWS
                if slab % OUT_CHUNK_SLABS == 0:
                    staging = stage_pool.tile([128, chunk_rt * SEG], F16, tag="sseg", bufs=2)
                xs = xslab_pool.tile([128, DCH, 2, SLAB_ROWS], F8, tag="xs", bufs=3)
                if slab == 0:
                    # Quarter the first slab across both queues: compute can
                    # start as soon as the first 256 rows + centroids land.
                    for q in range(4):
                        eng = nc.sync if q % 2 == 0 else nc.scalar
                        rq = q * (SLAB_ROWS // 4)
                        eng.dma_start(
                            out=xs[:, :, :, rq : rq + SLAB_ROWS // 4],
                            in_=xt_r[:, :, :, rq : rq + SLAB_ROWS // 4],
                        )
                else:
                    eng = nc.sync if slab % 2 == 0 else nc.scalar
                    eng.dma_start(
                        out=xs, in_=xt_r[:, :, :, r0 : r0 + SLAB_ROWS]
                    )
                for sub in range(SUBTILES):
                    mc = (slab % OUT_CHUNK_SLABS) * SUBTILES + sub
                    m0 = sub * 128
                    last = slab == N_SLABS - 1 and sub == SUBTILES - 1
                    # One 2-bank PSUM tile per row-tile: bank b holds the 512
                    # scores for clusters [512b, 512b+512).
                    ps = psum_pool.tile([128, 2, SEG_BANK, SEGW], F32, tag="ps")
                    # Interleave the two banks' matmuls, except for the very
                    # last row-tile: there, finish bank 0 first so its reduce
                    # overlaps bank 1's matmuls (shorter kernel tail).
                    banks_t = (
                        [(b, t) for b in range(2) for t in range(DCH)]
                        if last
                        else [(b, t) for t in range(DCH) for b in range(2)]
                    )
                    for b, t in banks_t:
                        nc.tensor.matmul(
                            ps[:, b, :, :],
                            xs[:, t, :, m0 : m0 + 128],
                            cb[:, t, :, 512 * b : 512 * b + 512],
                            start=(t == 0),
                            stop=(t == DCH - 1),
                            perf_mode=mybir.MatmulPerfMode.DoubleRow,
                        )
                    if last:
                        # Per-bank reduces: bank 0's runs under bank 1's MMs.
                        for b in range(2):
                            nc.vector.tensor_reduce(
                                staging[
                                    :,
                                    mc * SEG + b * SEG_BANK : mc * SEG
                                    + (b + 1) * SEG_BANK,
                                ],
                                ps[:, b, :, :],
                                axis=mybir.AxisListType.X,
                                op=mybir.AluOpType.max,
                            )
                    else:
                        # All 32 raw segment maxes in one DVE reduce over
                        # both PSUM banks, fp32 -> fp16.
                        nc.vector.tensor_reduce(
                            staging[:, mc * SEG : (mc + 1) * SEG],
                            ps,
                            axis=mybir.AxisListType.X,
                            op=mybir.AluOpType.max,
                        )
                if slab % OUT_CHUNK_SLABS == OUT_CHUNK_SLABS - 1:
                    m0c = (slab - OUT_CHUNK_SLABS + 1) * SUBTILES * SEG
                    nc.sync.dma_start(
                        out=out_seg[:, m0c : m0c + chunk_rt * SEG], in_=staging
                    )

    nc.finalize()
    return nc


def _get_program():
    global _PROGRAM
    if _PROGRAM is None:
        _PROGRAM = _build_program()
    return _PROGRAM


def _cluster_perm(cluster_centers):
    c2 = (cluster_centers.astype(np.float64) ** 2).sum(axis=1)
    return np.argsort(c2), c2


def _make_in_maps(features, cluster_centers):
    import ml_dtypes

    perm, _ = _cluster_perm(cluster_centers)
    cperm = cluster_centers[perm]
    f8 = features.astype(ml_dtypes.float8_e4m3)
    cbt = np.ascontiguousarray(cperm.T.astype(ml_dtypes.float8_e4m3))

    in_maps = []
    for i in range(N_CORES):
        shard = f8[i * ROWS_PER_CORE : (i + 1) * ROWS_PER_CORE]
        xtr = np.ascontiguousarray(shard.T)  # [768, 32768] fp8
        in_maps.append({"xt": xtr, "cbt": cbt})
    return in_maps


def _postprocess(res, features, cluster_centers):
    """Exact rescoring of every row over its candidate segments."""
    seg_parts = []
    for i in range(N_CORES):
        seg = res.results[i]["seg"]  # [128, 32*256] fp16
        seg_parts.append(
            seg.astype(np.float32).reshape(128, N_ROWTILES, SEG).transpose(1, 0, 2)
        )
    segraw = np.concatenate(seg_parts).reshape(N, SEG)

    perm, c2 = _cluster_perm(cluster_centers)
    cperm = cluster_centers[perm]
    bp = (-0.5 * c2[perm]).astype(np.float32)
    bmin = bp.reshape(SEG, SEGW).min(axis=1)
    bmax = bp.reshape(SEG, SEGW).max(axis=1)

    lb_top = (segraw + bmin).max(axis=1)
    close = (segraw + bmax) >= (lb_top - GAP_THRESHOLD)[:, None]

    best_val = np.full(N, -np.inf, np.float32)
    best_idx = np.zeros(N, np.int64)
    for s in range(SEG):
        rows = np.flatnonzero(close[:, s])
        if rows.size == 0:
            continue
        Cs = cperm[s * SEGW : (s + 1) * SEGW]
        bs = bp[s * SEGW : (s + 1) * SEGW]
        sc = features[rows] @ Cs.T + bs
        kl = sc.argmax(axis=1)
        v = sc[np.arange(rows.size), kl]
        upd = v > best_val[rows]
        rr = rows[upd]
        best_val[rr] = v[upd]
        best_idx[rr] = perm[s * SEGW + kl[upd]]
    return best_idx.astype(np.int32)


def kernel(features: np.ndarray, cluster_centers: np.ndarray) -> np.ndarray:
    from concourse.bass_utils import run_bass_kernel_spmd

    features = np.ascontiguousarray(features, dtype=np.float32)
    cluster_centers = np.ascontiguousarray(cluster_centers, dtype=np.float32)

    in_maps = _make_in_maps(features, cluster_centers)
    nc = _get_program()
    res = run_bass_kernel_spmd(nc, in_maps, core_ids=list(range(N_CORES)))
    return _postprocess(res, features, cluster_centers)


if __name__ == "__main__":
    rng = np.random.default_rng(0)
    f = rng.standard_normal((N, D)).astype(np.float32)
    c = rng.standard_normal((K, D)).astype(np.float32)
    got = kernel(f, c)
    d2 = (
        (f**2).sum(1, keepdims=True)
        - 2.0 * f @ c.T
        + (c**2).sum(1)
    )
    want = d2.argmin(1)
    print("mismatches:", (got != want).sum(), "/", N)


# revision 16
# speedup vs baseline: 1.0019x; 1.0019x over previous
"""KMeans inference (argmin over squared distances) on 8 Trainium2 cores.

Problem: features [262144, 768] fp32, cluster_centers [1024, 768] fp32.
Output: argmin_k ||x_i - c_k||^2 as int32 [262144].

Strategy (data-parallel over rows, fp8 scoring + exact host refinement):
  - argmin_k ||x-c_k||^2 == argmax_k (x.c_k - 0.5*||c_k||^2); the ||x||^2
    term is constant per row and drops out of the argmin.
  - Shard rows across 8 cores (32768 rows/core). Host pre-transposes each
    shard to xT [768, 32768] and quantizes to fp8 e4m3 (TRN FP8_EXP4 ==
    ml_dtypes.float8_e4m3; |x| ~ N(0,1) is far inside the +-240 range).
  - Per core: scores[m, k] = sum_d xT[d, m] * cT[d, k] via PE matmuls in
    fp8 with perf_mode=DoubleRow (2 fp8 weights per PE cell, contraction
    256 per matmul => 2x FLOP rate vs bf16/fp32r). d=768 = 3 chunks of
    256; both packed operands use the same d -> (chunk, pair, partition)
    layout so the contraction is consistent.
  - Clusters are pre-sorted by ||c||^2 on the host, so each of 32 segments
    of 32 clusters has a tight bias range [bmin_s, bmax_s]. The device
    exports only 32 raw (bias-free) per-segment score maxes per row: one
    DVE segmented reduce straight out of each PSUM bank, fp32 -> fp16.
    No scalar-engine copies, no bias add, no argmax index work on-device.
  - Host: segment s can contain the winner iff segraw_s + bmax_s >=
    max_s'(segraw_s' + bmin_s') - T. Rescore all such segments exactly
    (fp32 BLAS, grouped per segment) and take the exact argmax. fp8 score
    noise has sigma ~1.0, so T=5 makes a missed true-argmax essentially
    impossible (calibrated: 0 mismatches of 262144 at T>=4; ~2.3 segments
    rescored per row).
"""

import sys

sys.path.insert(0, "/opt/trn_rl_repo")

import numpy as np

N_CORES = 8
N, K, D = 262144, 1024, 768
ROWS_PER_CORE = N // N_CORES           # 32768
SLAB_ROWS = 1024                       # rows fetched per DMA slab
N_SLABS = ROWS_PER_CORE // SLAB_ROWS   # 32
SUBTILES = SLAB_ROWS // 128            # 8 row-tiles of 128 per slab
N_ROWTILES = ROWS_PER_CORE // 128      # 256
DCH = 3                                # d-chunks of 256 (DoubleRow pairs)
SEG = 32                               # exported segment maxes per row
SEGW = K // SEG                        # 32 clusters per segment
SEG_BANK = SEG // 2                    # segments per PSUM bank
OUT_CHUNK_SLABS = 1                    # stage flushed every slab
N_WARM_MM = 13                         # dummy matmuls to warm the PE clock

# fp8 e4m3 quantization of x and c gives score noise sigma ~1.0; the fp16
# segmax rounding adds <~0.07.  A missed true-argmax needs noise-diff > T
# across a segment boundary.  T=5 calibrates to 0/262144 mismatches while
# rescoring ~2.3 segments/row on the host.
GAP_THRESHOLD = 5.0

_PROGRAM = None


def _build_program():
    import concourse.mybir as mybir
    from concourse import bacc
    from concourse.tile import TileContext

    F32 = mybir.dt.float32
    F16 = mybir.dt.float16
    F8 = mybir.dt.float8e4

    nc = bacc.Bacc()
    # Inputs (per core): fp8-packed transposed feature shard + centroids
    # (row d ordering: d = t*256 + i*128 + p for chunk t, pair i, part p;
    # clusters pre-permuted by ||c||^2 on the host).
    xt = nc.declare_dram_parameter("xt", [D, ROWS_PER_CORE], F8, isOutput=False)
    cbt = nc.declare_dram_parameter("cbt", [D, K], F8, isOutput=False)
    # Output: seg[p, mc*SEG + s] = max_{k in seg s} x_row . c_k  (raw, no
    # bias) for row mc*128 + p.
    out_seg = nc.declare_dram_parameter(
        "seg", [128, SEG * N_ROWTILES], F16, isOutput=True
    )

    with TileContext(nc) as tc:
        with (
            tc.tile_pool(name="sbuf", bufs=1) as sbuf_pool,
            tc.tile_pool(name="psum", bufs=4, space="PSUM") as psum_pool,
        ):
            consts = xslab_pool = stage_pool = sbuf_pool
            cbt_r = cbt.rearrange("(t i p) k -> p t i k", p=128, i=2)
            xt_r = xt.rearrange("(t i p) r -> p t i r", p=128, i=2)

            # Warm the PE clock (HAM un-throttles after ~3.4us of sustained
            # activity) with dummy DoubleRow matmuls while the startup DMAs
            # land. The warm tile is filled by a small DMA (any finite fp8
            # data works - reuse the centroid table bytes) so no engine-side
            # memset / activation-table load sits on the critical path.
            warm = consts.tile([128, 2, 512], F8, tag="warm")
            nc.sync.dma_start(out=warm, in_=cbt_r[:, 0, :, 0:512])
            wps = psum_pool.tile([128, 2, SEG_BANK, SEGW], F32, tag="ps")
            for _ in range(N_WARM_MM):
                nc.tensor.matmul(
                    wps[:, 0, :, :],
                    warm[:, :, 0:128],
                    warm,
                    start=True,
                    stop=True,
                    perf_mode=mybir.MatmulPerfMode.DoubleRow,
                )

            # Centroids resident in SBUF, DoubleRow-packed: [128, 3, 2, 1024].
            # Split across both HWDGE queues (sync + scalar) by k-half so the
            # startup-critical load finishes in half the time.
            cb = consts.tile([128, DCH, 2, K], F8, tag="cb")
            nc.sync.dma_start(out=cb[:, :, :, 0:512], in_=cbt_r[:, :, :, 0:512])
            nc.scalar.dma_start(out=cb[:, :, :, 512:1024], in_=cbt_r[:, :, :, 512:1024])

            chunk_rt = OUT_CHUNK_SLABS * SUBTILES  # 16 row-tiles per chunk
            staging = None

            for slab in range(N_SLABS):
                r0 = slab * SLAB_ROWS
                if slab % OUT_CHUNK_SLABS == 0:
                    staging = stage_pool.tile([128, chunk_rt * SEG], F16, tag="sseg", bufs=2)
                xs = xslab_pool.tile([128, DCH, 2, SLAB_ROWS], F8, tag="xs", bufs=3)
                if slab == 0:
                    # Quarter the first slab across both queues: compute can
                    # start as soon as the first 256 rows + centroids land.
                    for q in range(4):
                        eng = nc.sync if q % 2 == 0 else nc.scalar
                        rq = q * (SLAB_ROWS // 4)
                        eng.dma_start(
                            out=xs[:, :, :, rq : rq + SLAB_ROWS // 4],
                            in_=xt_r[:, :, :, rq : rq + SLAB_ROWS // 4],
                        )
                else:
                    eng = nc.sync if slab % 2 == 0 else nc.scalar
                    eng.dma_start(
                        out=xs, in_=xt_r[:, :, :, r0 : r0 + SLAB_ROWS]
                    )
                for sub in range(SUBTILES):
                    mc = (slab % OUT_CHUNK_SLABS) * SUBTILES + sub
                    m0 = sub * 128
                    last = slab == N_SLABS - 1 and sub == SUBTILES - 1
                    # One 2-bank PSUM tile per row-tile: bank b holds the 512
                    # scores for clusters [512b, 512b+512).
                    ps = psum_pool.tile([128, 2, SEG_BANK, SEGW], F32, tag="ps")
                    # Interleave the two banks' matmuls, except for the very
                    # last row-tile: there, finish bank 0 first so its reduce
                    # overlaps bank 1's matmuls (shorter kernel tail).
                    banks_t = (
                        [(b, t) for b in range(2) for t in range(DCH)]
                        if last
                        else [(b, t) for t in range(DCH) for b in range(2)]
                    )
                    for b, t in banks_t:
                        nc.tensor.matmul(
                            ps[:, b, :, :],
                            xs[:, t, :, m0 : m0 + 128],
                            cb[:, t, :, 512 * b : 512 * b + 512],
                            start=(t == 0),
                            stop=(t == DCH - 1),
                            perf_mode=mybir.MatmulPerfMode.DoubleRow,
                        )
                    if last:
                        # Per-bank reduces: bank 0's runs under bank 1's MMs.
                        for b in range(2):
                            nc.vector.tensor_reduce(
                                staging[
                                    :,
                                    mc * SEG + b * SEG_BANK : mc * SEG
                                    + (b + 1) * SEG_BANK,
                                ],
                                ps[:, b, :, :],
                                axis=mybir.AxisListType.X,
                                op=mybir.AluOpType.max,
                            )
                    else:
                        # All 32 raw segment maxes in one DVE reduce over
                        # both PSUM banks, fp32 -> fp16.
                        nc.vector.tensor_reduce(
                            staging[:, mc * SEG : (mc + 1) * SEG],
                            ps,
                            axis=mybir.AxisListType.X,
                            op=mybir.AluOpType.max,
                        )
                if slab % OUT_CHUNK_SLABS == OUT_CHUNK_SLABS - 1:
                    m0c = (slab - OUT_CHUNK_SLABS + 1) * SUBTILES * SEG
                    nc.sync.dma_start(
                        out=out_seg[:, m0c : m0c + chunk_rt * SEG], in_=staging
                    )

    nc.finalize()
    return nc


def _get_program():
    global _PROGRAM
    if _PROGRAM is None:
        _PROGRAM = _build_program()
    return _PROGRAM


def _cluster_perm(cluster_centers):
    c2 = (cluster_centers.astype(np.float64) ** 2).sum(axis=1)
    return np.argsort(c2), c2


def _make_in_maps(features, cluster_centers):
    import ml_dtypes

    perm, _ = _cluster_perm(cluster_centers)
    cperm = cluster_centers[perm]
    f8 = features.astype(ml_dtypes.float8_e4m3)
    cbt = np.ascontiguousarray(cperm.T.astype(ml_dtypes.float8_e4m3))

    in_maps = []
    for i in range(N_CORES):
        shard = f8[i * ROWS_PER_CORE : (i + 1) * ROWS_PER_CORE]
        xtr = np.ascontiguousarray(shard.T)  # [768, 32768] fp8
        in_maps.append({"xt": xtr, "cbt": cbt})
    return in_maps


def _postprocess(res, features, cluster_centers):
    """Exact rescoring of every row over its candidate segments."""
    seg_parts = []
    for i in range(N_CORES):
        seg = res.results[i]["seg"]  # [128, 32*256] fp16
        seg_parts.append(
            seg.astype(np.float32).reshape(128, N_ROWTILES, SEG).transpose(1, 0, 2)
        )
    segraw = np.concatenate(seg_parts).reshape(N, SEG)

    perm, c2 = _cluster_perm(cluster_centers)
    cperm = cluster_centers[perm]
    bp = (-0.5 * c2[perm]).astype(np.float32)
    bmin = bp.reshape(SEG, SEGW).min(axis=1)
    bmax = bp.reshape(SEG, SEGW).max(axis=1)

    lb_top = (segraw + bmin).max(axis=1)
    close = (segraw + bmax) >= (lb_top - GAP_THRESHOLD)[:, None]

    best_val = np.full(N, -np.inf, np.float32)
    best_idx = np.zeros(N, np.int64)
    for s in range(SEG):
        rows = np.flatnonzero(close[:, s])
        if rows.size == 0:
            continue
        Cs = cperm[s * SEGW : (s + 1) * SEGW]
        bs = bp[s * SEGW : (s + 1) * SEGW]
        sc = features[rows] @ Cs.T + bs
        kl = sc.argmax(axis=1)
        v = sc[np.arange(rows.size), kl]
        upd = v > best_val[rows]
        rr = rows[upd]
        best_val[rr] = v[upd]
        best_idx[rr] = perm[s * SEGW + kl[upd]]
    return best_idx.astype(np.int32)


def kernel(features: np.ndarray, cluster_centers: np.ndarray) -> np.ndarray:
    from concourse.bass_utils import run_bass_kernel_spmd

    features = np.ascontiguousarray(features, dtype=np.float32)
    cluster_centers = np.ascontiguousarray(cluster_centers, dtype=np.float32)

    in_maps = _make_in_maps(features, cluster_centers)
    nc = _get_program()
    res = run_bass_kernel_spmd(nc, in_maps, core_ids=list(range(N_CORES)))
    return _postprocess(res, features, cluster_centers)


if __name__ == "__main__":
    rng = np.random.default_rng(0)
    f = rng.standard_normal((N, D)).astype(np.float32)
    c = rng.standard_normal((K, D)).astype(np.float32)
    got = kernel(f, c)
    d2 = (
        (f**2).sum(1, keepdims=True)
        - 2.0 * f @ c.T
        + (c**2).sum(1)
    )
    want = d2.argmin(1)
    print("mismatches:", (got != want).sum(), "/", N)


# revision 18
# speedup vs baseline: 1.0060x; 1.0042x over previous
"""KMeans inference (argmin over squared distances) on 8 Trainium2 cores.

Problem: features [262144, 768] fp32, cluster_centers [1024, 768] fp32.
Output: argmin_k ||x_i - c_k||^2 as int32 [262144].

Strategy (data-parallel over rows, fp8 scoring + exact host refinement):
  - argmin_k ||x-c_k||^2 == argmax_k (x.c_k - 0.5*||c_k||^2); the ||x||^2
    term is constant per row and drops out of the argmin.
  - Shard rows across 8 cores (32768 rows/core). Host pre-transposes each
    shard to xT [768, 32768] and quantizes to fp8 e4m3 (TRN FP8_EXP4 ==
    ml_dtypes.float8_e4m3; |x| ~ N(0,1) is far inside the +-240 range).
  - Per core: scores[m, k] = sum_d xT[d, m] * cT[d, k] via PE matmuls in
    fp8 with perf_mode=DoubleRow (2 fp8 weights per PE cell, contraction
    256 per matmul => 2x FLOP rate vs bf16/fp32r). d=768 = 3 chunks of
    256; both packed operands use the same d -> (chunk, pair, partition)
    layout so the contraction is consistent.
  - Clusters are pre-sorted by ||c||^2 on the host, so each of 32 segments
    of 32 clusters has a tight bias range [bmin_s, bmax_s]. The device
    exports only 32 raw (bias-free) per-segment score maxes per row: one
    DVE segmented reduce straight out of each PSUM bank, fp32 -> fp16.
    No scalar-engine copies, no bias add, no argmax index work on-device.
  - Host: segment s can contain the winner iff segraw_s + bmax_s >=
    max_s'(segraw_s' + bmin_s') - T. Rescore all such segments exactly
    (fp32 BLAS, grouped per segment) and take the exact argmax. fp8 score
    noise has sigma ~1.0, so T=5 makes a missed true-argmax essentially
    impossible (calibrated: 0 mismatches of 262144 at T>=4; ~2.3 segments
    rescored per row).
"""

import sys

sys.path.insert(0, "/opt/trn_rl_repo")

import numpy as np

N_CORES = 8
N, K, D = 262144, 1024, 768
ROWS_PER_CORE = N // N_CORES           # 32768
SLAB_ROWS = 1024                       # rows fetched per DMA slab
N_SLABS = ROWS_PER_CORE // SLAB_ROWS   # 32
SUBTILES = SLAB_ROWS // 128            # 8 row-tiles of 128 per slab
N_ROWTILES = ROWS_PER_CORE // 128      # 256
DCH = 3                                # d-chunks of 256 (DoubleRow pairs)
SEG = 32                               # exported segment maxes per row
SEGW = K // SEG                        # 32 clusters per segment
SEG_BANK = SEG // 2                    # segments per PSUM bank
OUT_CHUNK_SLABS = 2                    # stage flushed every 2 slabs
N_WARM_MM = 14                         # dummy matmuls to warm the PE clock

# fp8 e4m3 quantization of x and c gives score noise sigma ~1.0; the fp16
# segmax rounding adds <~0.07.  A missed true-argmax needs noise-diff > T
# across a segment boundary.  T=5 calibrates to 0/262144 mismatches while
# rescoring ~2.3 segments/row on the host.
GAP_THRESHOLD = 5.0

_PROGRAM = None


def _build_program():
    import concourse.mybir as mybir
    from concourse import bacc
    from concourse.tile import TileContext

    F32 = mybir.dt.float32
    F16 = mybir.dt.float16
    F8 = mybir.dt.float8e4

    nc = bacc.Bacc()
    # Inputs (per core): fp8-packed transposed feature shard + centroids
    # (row d ordering: d = t*256 + i*128 + p for chunk t, pair i, part p;
    # clusters pre-permuted by ||c||^2 on the host).
    xt = nc.declare_dram_parameter("xt", [D, ROWS_PER_CORE], F8, isOutput=False)
    cbt = nc.declare_dram_parameter("cbt", [D, K], F8, isOutput=False)
    # Output: seg[p, mc*SEG + s] = max_{k in seg s} x_row . c_k  (raw, no
    # bias) for row mc*128 + p.
    out_seg = nc.declare_dram_parameter(
        "seg", [128, SEG * N_ROWTILES], F16, isOutput=True
    )

    with TileContext(nc) as tc:
        with (
            tc.tile_pool(name="sbuf", bufs=1) as sbuf_pool,
            tc.tile_pool(name="psum", bufs=4, space="PSUM") as psum_pool,
        ):
            consts = xslab_pool = stage_pool = sbuf_pool
            cbt_r = cbt.rearrange("(t i p) k -> p t i k", p=128, i=2)
            xt_r = xt.rearrange("(t i p) r -> p t i r", p=128, i=2)

            # Warm the PE clock (HAM un-throttles after ~3.4us of sustained
            # activity) with dummy DoubleRow matmuls while the startup DMA
            # transfers land (~12.5us: 768KB of centroids + first rows over
            # two queues). gpsimd memset is the fastest warm-tile init -
            # engine-side, no DMA completion latency, no ACT table load.
            warm = consts.tile([128, 2, 512], F8, tag="warm")
            nc.gpsimd.memset(warm, 0)
            wps = psum_pool.tile([128, 2, SEG_BANK, SEGW], F32, tag="ps")
            for _ in range(N_WARM_MM):
                nc.tensor.matmul(
                    wps[:, 0, :, :],
                    warm[:, :, 0:128],
                    warm,
                    start=True,
                    stop=True,
                    perf_mode=mybir.MatmulPerfMode.DoubleRow,
                )

            # Centroids resident in SBUF, DoubleRow-packed: [128, 3, 2, 1024].
            # Split across both HWDGE queues (sync + scalar) by k-half so the
            # startup-critical load finishes in half the time.
            cb = consts.tile([128, DCH, 2, K], F8, tag="cb")
            nc.sync.dma_start(out=cb[:, :, :, 0:512], in_=cbt_r[:, :, :, 0:512])
            nc.scalar.dma_start(out=cb[:, :, :, 512:1024], in_=cbt_r[:, :, :, 512:1024])

            chunk_rt = OUT_CHUNK_SLABS * SUBTILES  # 16 row-tiles per chunk
            staging = None

            for slab in range(N_SLABS):
                r0 = slab * SLAB_ROWS
                if slab % OUT_CHUNK_SLABS == 0:
                    staging = stage_pool.tile([128, chunk_rt * SEG], F16, tag="sseg", bufs=2)
                xs = xslab_pool.tile([128, DCH, 2, SLAB_ROWS], F8, tag="xs", bufs=3)
                if slab == 0:
                    # Quarter the first slab across both queues: compute can
                    # start as soon as the first 256 rows + centroids land.
                    for q in range(4):
                        eng = nc.sync if q % 2 == 0 else nc.scalar
                        rq = q * (SLAB_ROWS // 4)
                        eng.dma_start(
                            out=xs[:, :, :, rq : rq + SLAB_ROWS // 4],
                            in_=xt_r[:, :, :, rq : rq + SLAB_ROWS // 4],
                        )
                else:
                    eng = nc.sync if slab % 2 == 0 else nc.scalar
                    eng.dma_start(
                        out=xs, in_=xt_r[:, :, :, r0 : r0 + SLAB_ROWS]
                    )
                for sub in range(SUBTILES):
                    mc = (slab % OUT_CHUNK_SLABS) * SUBTILES + sub
                    m0 = sub * 128
                    last = slab == N_SLABS - 1 and sub == SUBTILES - 1
                    # One 2-bank PSUM tile per row-tile: bank b holds the 512
                    # scores for clusters [512b, 512b+512).
                    ps = psum_pool.tile([128, 2, SEG_BANK, SEGW], F32, tag="ps")
                    # Interleave the two banks' matmuls, except for the very
                    # last row-tile: there, finish bank 0 first so its reduce
                    # overlaps bank 1's matmuls (shorter kernel tail).
                    banks_t = (
                        [(b, t) for b in range(2) for t in range(DCH)]
                        if last
                        else [(b, t) for t in range(DCH) for b in range(2)]
                    )
                    for b, t in banks_t:
                        nc.tensor.matmul(
                            ps[:, b, :, :],
                            xs[:, t, :, m0 : m0 + 128],
                            cb[:, t, :, 512 * b : 512 * b + 512],
                            start=(t == 0),
                            stop=(t == DCH - 1),
                            perf_mode=mybir.MatmulPerfMode.DoubleRow,
                        )
                    if last:
                        # Per-bank reduces: bank 0's runs under bank 1's MMs.
                        for b in range(2):
                            nc.vector.tensor_reduce(
                                staging[
                                    :,
                                    mc * SEG + b * SEG_BANK : mc * SEG
                                    + (b + 1) * SEG_BANK,
                                ],
                                ps[:, b, :, :],
                                axis=mybir.AxisListType.X,
                                op=mybir.AluOpType.max,
                            )
                    else:
                        # All 32 raw segment maxes in one DVE reduce over
                        # both PSUM banks, fp32 -> fp16.
                        nc.vector.tensor_reduce(
                            staging[:, mc * SEG : (mc + 1) * SEG],
                            ps,
                            axis=mybir.AxisListType.X,
                            op=mybir.AluOpType.max,
                        )
                if slab % OUT_CHUNK_SLABS == OUT_CHUNK_SLABS - 1:
                    m0c = (slab - OUT_CHUNK_SLABS + 1) * SUBTILES * SEG
                    nc.sync.dma_start(
                        out=out_seg[:, m0c : m0c + chunk_rt * SEG], in_=staging
                    )

    nc.finalize()
    return nc


def _get_program():
    global _PROGRAM
    if _PROGRAM is None:
        _PROGRAM = _build_program()
    return _PROGRAM


def _cluster_perm(cluster_centers):
    c2 = (cluster_centers.astype(np.float64) ** 2).sum(axis=1)
    return np.argsort(c2), c2


def _make_in_maps(features, cluster_centers):
    import ml_dtypes

    perm, _ = _cluster_perm(cluster_centers)
    cperm = cluster_centers[perm]
    f8 = features.astype(ml_dtypes.float8_e4m3)
    cbt = np.ascontiguousarray(cperm.T.astype(ml_dtypes.float8_e4m3))

    in_maps = []
    for i in range(N_CORES):
        shard = f8[i * ROWS_PER_CORE : (i + 1) * ROWS_PER_CORE]
        xtr = np.ascontiguousarray(shard.T)  # [768, 32768] fp8
        in_maps.append({"xt": xtr, "cbt": cbt})
    return in_maps


def _postprocess(res, features, cluster_centers):
    """Exact rescoring of every row over its candidate segments."""
    seg_parts = []
    for i in range(N_CORES):
        seg = res.results[i]["seg"]  # [128, 32*256] fp16
        seg_parts.append(
            seg.astype(np.float32).reshape(128, N_ROWTILES, SEG).transpose(1, 0, 2)
        )
    segraw = np.concatenate(seg_parts).reshape(N, SEG)

    perm, c2 = _cluster_perm(cluster_centers)
    cperm = cluster_centers[perm]
    bp = (-0.5 * c2[perm]).astype(np.float32)
    bmin = bp.reshape(SEG, SEGW).min(axis=1)
    bmax = bp.reshape(SEG, SEGW).max(axis=1)

    lb_top = (segraw + bmin).max(axis=1)
    close = (segraw + bmax) >= (lb_top - GAP_THRESHOLD)[:, None]

    best_val = np.full(N, -np.inf, np.float32)
    best_idx = np.zeros(N, np.int64)
    for s in range(SEG):
        rows = np.flatnonzero(close[:, s])
        if rows.size == 0:
            continue
        Cs = cperm[s * SEGW : (s + 1) * SEGW]
        bs = bp[s * SEGW : (s + 1) * SEGW]
        sc = features[rows] @ Cs.T + bs
        kl = sc.argmax(axis=1)
        v = sc[np.arange(rows.size), kl]
        upd = v > best_val[rows]
        rr = rows[upd]
        best_val[rr] = v[upd]
        best_idx[rr] = perm[s * SEGW + kl[upd]]
    return best_idx.astype(np.int32)


def kernel(features: np.ndarray, cluster_centers: np.ndarray) -> np.ndarray:
    from concourse.bass_utils import run_bass_kernel_spmd

    features = np.ascontiguousarray(features, dtype=np.float32)
    cluster_centers = np.ascontiguousarray(cluster_centers, dtype=np.float32)

    in_maps = _make_in_maps(features, cluster_centers)
    nc = _get_program()
    res = run_bass_kernel_spmd(nc, in_maps, core_ids=list(range(N_CORES)))
    return _postprocess(res, features, cluster_centers)


if __name__ == "__main__":
    rng = np.random.default_rng(0)
    f = rng.standard_normal((N, D)).astype(np.float32)
    c = rng.standard_normal((K, D)).astype(np.float32)
    got = kernel(f, c)
    d2 = (
        (f**2).sum(1, keepdims=True)
        - 2.0 * f @ c.T
        + (c**2).sum(1)
    )
    want = d2.argmin(1)
    print("mismatches:", (got != want).sum(), "/", N)
